# revision 3
# baseline (speedup 1.0000x reference)
"""Sparse masked attention on 8 TRN2 NeuronCores.

reference:  O = softmax((Q K^T * mq[:,None] + log(mk[None,:])) / 8) @ V
  - rows with mq=0: scores all equal -> uniform average of V over mk=1 keys
  - keys with mk=0: exactly dropped from the softmax

Strategy: batch (b=8) is data-parallel across the 8 cores. On the host we
compact each batch to its unmasked queries/keys (~n/2 each), so each core
computes a dense ~2176x2176 attention instead of 4096x4096:

  S^T[mpair, n] = (Kc^T pair).T @ Qc^T       (TensorE bf16, d=64, row-group
                                              pairs: two chunks concurrently,
                                              into one 2-bank PSUM tile)
  P^T = exp(S^T / 8) -> bf16                 (one fused instruction per pair:
                                              ScalarE exact exp / VectorE
                                              Schraudolph fast-exp, alternating)
  O^T[65, n]    += Vext[mchunk].T @ P^T      (TensorE bf16, 64-row contraction
                                              halves on alternating row groups
                                              -> olo/ohi PSUM accumulators;
                                              the rg interleave lets every
                                              LDWEIGHTS hide under the other
                                              row group's matmul)

where Vext = [V | 1]: the ones column accumulates the softmax denominator.
No row-max subtraction is needed: scores/8 ~ N(0,1), exp stays in range.
olo/ohi are copied out separately (ScalarE/VectorE in parallel) and summed
on the host, which also divides by the denominator, scatters rows back, and
fills masked query rows with mean(V[mk=1]).

A warmup spin of matmuls runs during the input-DMA wait so the PE's HAM
clock gate is already at 2.4 GHz when the real matmuls start.
"""

import numpy as np
import ml_dtypes

N_CORES = 8
W = 512  # n-block width (PSUM bank / fp32-accum matmul free-dim limit)
N_WARM = 9    # warmup matmuls (HAM clock-gate spin during DMA wait)
NW_COLS = 512  # moving columns per warmup matmul


def _round_up(x, mult):
    return ((x + mult - 1) // mult) * mult


_build_cache = {}


def _build(ncap, mcap):
    """Per-core graph. Inputs (per core):
      qt2  [128, ncap]           bf16   Q^T duplicated: rows 0-63 == rows 64-127
      ktp  [128, npairs*128]     bf16   K^T chunk pairs: pair p = chunk 2p on
                                        partitions 0-63, chunk 2p+1 on 64-127
      vext [128, mchunks*65]     bf16   partition-major Vext chunks: partition
                                        r, cols [c*65:(c+1)*65] = Vext row
                                        c*128+r = [V row | 1.0] (0 if padding)
    Output: out [nblocks*130, W] f32: block j rows [130j:130j+65] = lo half,
      rows [130j+65:130j+130] = hi half; cols [:w_j] = partial
      [ O^T numerator (64 rows) ; denominator (1 row) ] for n-cols j*W..+w_j.
      The host sums lo+hi.
    """
    key = (ncap, mcap)
    if key in _build_cache:
        return _build_cache[key]

    import concourse.bacc as bacc
    import concourse.mybir as mybir
    import concourse.tile as tile

    f32 = mybir.dt.float32
    bf16 = mybir.dt.bfloat16
    i16 = mybir.dt.int16
    mchunks = mcap // 128
    npairs = (mchunks + 1) // 2
    # balanced n-blocks, each <= W (PSUM bank limit)
    nblocks = (ncap + W - 1) // W
    base, extra = divmod(ncap, nblocks)
    widths = [base + (1 if i < extra else 0) for i in range(nblocks)]
    blocks, off = [], 0
    for wd in widths:
        blocks.append((off, wd))
        off += wd
    EXP = mybir.ActivationFunctionType.Exp

    # Schraudolph fast exp on bf16 bit pattern, via int16:
    #   i16 = (int16)(s * (2^7/ln2)/8 + (127*2^7 - C)) ; bitcast -> bf16
    FEXP_A = float(2 ** 7 / np.log(2.0) / 8.0)
    FEXP_B = float(127 * 2 ** 7 - 7.5)

    nc = bacc.Bacc("TRN2", target_bir_lowering=False, debug=False,
                   num_devices=N_CORES)
    qt_d = nc.dram_tensor("qt2", [128, ncap], bf16, kind="ExternalInput")
    ktp_d = nc.dram_tensor("ktp", [128, npairs * 128], bf16,
                           kind="ExternalInput")
    vext_d = nc.dram_tensor("vext", [128, mchunks * 65], bf16,
                            kind="ExternalInput")
    out_d = nc.dram_tensor("out", [len(blocks) * 130, W], f32,
                           kind="ExternalOutput")

    with tile.TileContext(nc) as tc:
        with (
            tc.tile_pool(name="resident", bufs=1) as resident,
            tc.tile_pool(name="pt", bufs=4) as ptp,
            tc.tile_pool(name="osb", bufs=2) as osbp,
            tc.tile_pool(name="psum", bufs=1, space="PSUM") as psum,
        ):
            # sliced DMAs: block-0 slices first so compute starts early
            kt_sb = resident.tile([128, npairs * 128], bf16)
            kcut = min(128, npairs * 128)
            nc.sync.dma_start(kt_sb[:, 0:kcut], ktp_d[:, 0:kcut])
            w0 = blocks[0][1]
            qt_sb = resident.tile([128, ncap], bf16)
            nc.sync.dma_start(qt_sb[:, 0:w0], qt_d[:, 0:w0])
            v_sb = resident.tile([128, mchunks * 65], bf16)
            vcut = min(130, mchunks * 65)
            nc.sync.dma_start(v_sb[:, 0:vcut], vext_d[:, 0:vcut])
            if kcut < npairs * 128:
                nc.sync.dma_start(kt_sb[:, kcut:], ktp_d[:, kcut:])
            if vcut < mchunks * 65:
                nc.sync.dma_start(v_sb[:, vcut:], vext_d[:, vcut:])
            if ncap > w0:
                nc.sync.dma_start(qt_sb[:, w0:], qt_d[:, w0:])

            # PE warmup: keep the HAM activity monitor busy while input DMAs
            # land, so real matmuls run at 2.4 GHz from the start.
            wscr = resident.tile([64, NW_COLS], bf16)
            nc.gpsimd.memset(wscr[:, :], 0)
            for _ in range(N_WARM):
                stw = psum.tile([128, 2, W], f32, tag="st", bufs=3)
                nc.tensor.matmul(stw[0:64, 0, 0:NW_COLS], wscr[:, 0:64],
                                 wscr[:, :], start=True, stop=True,
                                 tile_position=(0, 0))

            pending_drain = None  # (jb, olo, ohi, w) of the previous block

            def drain(jb, olo, ohi, w):
                osl = osbp.tile([65, W], f32, tag="osl")
                nc.vector.tensor_copy(osl[:, 0:w], olo[:, 0:w])
                osh = osbp.tile([65, W], f32, tag="osh")
                nc.scalar.copy(osh[:, 0:w], ohi[:, 0:w])
                nc.sync.dma_start(out_d[jb * 130:jb * 130 + 65, 0:w],
                                  osl[:, 0:w])
                nc.sync.dma_start(out_d[jb * 130 + 65:jb * 130 + 130, 0:w],
                                  osh[:, 0:w])

            def mov(t, half, lo, hi, w):
                ap = t[lo:hi, half, 0:w]
                return ap.bitcast(bf16) if t.dtype == i16 else ap

            parity = 0
            for jb, (j0, w) in enumerate(blocks):
                olo = psum.tile([65, W], f32, tag="olo", bufs=1)
                ohi = psum.tile([65, W], f32, tag="ohi", bufs=1)
                prev = None  # (pt tile, p, has_b)

                def pv(t, p, has_b, last):
                    # 64-row contraction halves on alternating row groups:
                    # rows 0-63 accumulate into olo at (0,0), rows 64-127
                    # into ohi at (64,0)
                    for half, mi in ((0, 2 * p), (1, 2 * p + 1)):
                        if half == 1 and not has_b:
                            continue
                        lastc = last and mi == mchunks - 1
                        nc.tensor.matmul(
                            olo[:, 0:w], v_sb[0:64, mi * 65:(mi + 1) * 65],
                            mov(t, half, 0, 64, w), start=(mi == 0),
                            stop=lastc, tile_position=(0, 0),
                            skip_group_check=True)
                        nc.tensor.matmul(
                            ohi[:, 0:w], v_sb[64:128, mi * 65:(mi + 1) * 65],
                            mov(t, half, 64, 128, w), start=(mi == 0),
                            stop=lastc, tile_position=(64, 0),
                            skip_group_check=True)

                for p in range(npairs):
                    has_b = 2 * p + 1 < mchunks
                    # S^T for two m-chunks concurrently (PE row groups) into
                    # one 2-bank PSUM tile
                    st = psum.tile([128, 2, W], f32, tag="st", bufs=3)
                    nc.tensor.matmul(
                        st[:, 0, 0:w], kt_sb[0:64, p * 128:(p + 1) * 128],
                        qt_sb[0:64, j0:j0 + w],
                        start=True, stop=True, tile_position=(0, 0))
                    if has_b:
                        nc.tensor.matmul(
                            st[:, 1, 0:w], kt_sb[64:128, p * 128:(p + 1) * 128],
                            qt_sb[64:128, j0:j0 + w],
                            start=True, stop=True, tile_position=(64, 0))
                    # previous block's output drain: issue before this block's
                    # first exp so olo/ohi free up for pv(pair 0) below
                    if p == 0 and pending_drain is not None:
                        drain(*pending_drain)
                        pending_drain = None
                    # previous pair's PV matmuls go here so the PE has work
                    # queued ahead of the exp-dependent ones
                    if prev is not None:
                        pv(*prev, last=False)
                    # P^T = exp(S^T/8), one fused instruction per pair:
                    # exact on ScalarE / fast-exp on VectorE, alternating
                    use_act = (p % 2 == parity)
                    nh = 2 if has_b else 1
                    if use_act:
                        t = ptp.tile([128, 2, W], bf16, tag="pt")
                        nc.scalar.activation(t[:, 0:nh, 0:w], st[:, 0:nh, 0:w],
                                             EXP, scale=0.125)
                    else:
                        t = ptp.tile([128, 2, W], i16, tag="pt")
                        nc.vector.tensor_scalar(
                            t[:, 0:nh, 0:w], st[:, 0:nh, 0:w], FEXP_A, FEXP_B,
                            mybir.AluOpType.mult, mybir.AluOpType.add)
                    prev = (t, p, has_b)
                pv(*prev, last=True)
                pending_drain = (jb, olo, ohi, w)
                parity ^= 1
            drain(*pending_drain)

    nc.compile()
    _build_cache[key] = nc
    return nc


def _run(inputs, trace=False):
    queries = np.asarray(inputs["queries"], dtype=np.float32)
    keys = np.asarray(inputs["keys"], dtype=np.float32)
    values = np.asarray(inputs["values"], dtype=np.float32)
    mask_query = np.asarray(inputs["mask_query"])
    mask_key = np.asarray(inputs["mask_key"])

    b, n, d = queries.shape
    dv = values.shape[2]
    assert b == N_CORES, f"batch {b} != {N_CORES} cores"
    bf = ml_dtypes.bfloat16

    idx_q = [np.flatnonzero(mask_query[i]) for i in range(b)]
    idx_k = [np.flatnonzero(mask_key[i]) for i in range(b)]
    ncap = max(max(len(ix) for ix in idx_q), 64)
    mcap = _round_up(max(max(len(ix) for ix in idx_k), 1), 128)
    mchunks = mcap // 128
    npairs = (mchunks + 1) // 2
    nblocks = (ncap + W - 1) // W
    base, extra = divmod(ncap, nblocks)
    bwidths = [base + (1 if i < extra else 0) for i in range(nblocks)]

    qt2 = np.zeros((b, 128, ncap), bf)
    ktp = np.zeros((b, 128, npairs * 128), bf)
    vext = np.zeros((b, 128, mchunks * 65), bf)
    for i in range(b):
        nq, nk = len(idx_q[i]), len(idx_k[i])
        qt2[i, 0:64, :nq] = queries[i, idx_q[i]].T.astype(bf)
        qt2[i, 64:128, :] = qt2[i, 0:64, :]
        kc_t = np.zeros((64, mcap), np.float32)
        kc_t[:, :nk] = keys[i, idx_k[i]].T
        kc_t = kc_t.astype(bf)
        for p in range(npairs):
            ktp[i, 0:64, p * 128:(p + 1) * 128] = \
                kc_t[:, (2 * p) * 128:(2 * p + 1) * 128]
            if 2 * p + 1 < mchunks:
                ktp[i, 64:128, p * 128:(p + 1) * 128] = \
                    kc_t[:, (2 * p + 1) * 128:(2 * p + 2) * 128]
        ve = np.zeros((mcap, 65), np.float32)
        ve[:nk, :dv] = values[i, idx_k[i]]
        ve[:nk, dv] = 1.0
        # partition-major: [chunk, row] -> [row_in_chunk, chunk*65+col]
        vext[i] = ve.reshape(mchunks, 128, 65).transpose(1, 0, 2) \
                    .reshape(128, mchunks * 65).astype(bf)

    nc = _build(ncap, mcap)

    from concourse.bass_utils import run_bass_kernel_spmd
    in_maps = [{"qt2": qt2[i], "ktp": ktp[i], "vext": vext[i]}
               for i in range(b)]
    res = run_bass_kernel_spmd(nc, in_maps, core_ids=list(range(N_CORES)),
                               trace=trace)

    out = np.empty((b, n, dv), np.float32)
    for i in range(b):
        ot = res.results[i]["out"]  # [nblocks*130, W]
        nq, nk = len(idx_q[i]), len(idx_k[i])
        full = np.concatenate(
            [ot[jb * 130:jb * 130 + 65, :bwidths[jb]]
             + ot[jb * 130 + 65:jb * 130 + 130, :bwidths[jb]]
             for jb in range(nblocks)], axis=1)
        num = full[:dv, :nq]
        den = full[dv, :nq]
        if nk > 0:
            out[i, :, :] = values[i, idx_k[i]].mean(axis=0)
        else:
            out[i, :, :] = 0.0
        if nq > 0:
            out[i, idx_q[i], :] = (num / den).T
    return out, res


def kernel(**inputs):
    out, _ = _run(inputs, trace=False)
    return out


# revision 4
# speedup vs baseline: 1.0208x; 1.0208x over previous
"""Sparse masked attention on 8 TRN2 NeuronCores.

reference:  O = softmax((Q K^T * mq[:,None] + log(mk[None,:])) / 8) @ V
  - rows with mq=0: scores all equal -> uniform average of V over mk=1 keys
  - keys with mk=0: exactly dropped from the softmax

Strategy: batch (b=8) is data-parallel across the 8 cores. On the host we
compact each batch to its unmasked queries/keys (~n/2 each), so each core
computes a dense ~2176x2176 attention instead of 4096x4096:

  S^T[mpair, n] = (Kc^T pair).T @ Qc^T       (TensorE bf16, d=64, row-group
                                              pairs: two chunks concurrently,
                                              into one 2-bank PSUM tile)
  P^T = exp(S^T / 8) -> bf16                 (one fused instruction per pair:
                                              ScalarE exact exp / VectorE
                                              Schraudolph fast-exp, alternating)
  O^T[65, n]    += Vext[mchunk].T @ P^T      (TensorE bf16, 64-row contraction
                                              halves on alternating row groups
                                              -> olo/ohi PSUM accumulators;
                                              the rg interleave lets every
                                              LDWEIGHTS hide under the other
                                              row group's matmul)

where Vext = [V | 1]: the ones column accumulates the softmax denominator.
No row-max subtraction is needed: scores/8 ~ N(0,1), exp stays in range.
olo/ohi are copied out separately (ScalarE/VectorE in parallel) and summed
on the host, which also divides by the denominator, scatters rows back, and
fills masked query rows with mean(V[mk=1]).

A warmup spin of matmuls runs during the input-DMA wait so the PE's HAM
clock gate is already at 2.4 GHz when the real matmuls start.
"""

import numpy as np
import ml_dtypes

N_CORES = 8
W = 512  # n-block width (PSUM bank / fp32-accum matmul free-dim limit)
N_WARM = 9    # warmup matmuls (HAM clock-gate spin during DMA wait)
NW_COLS = 512  # moving columns per warmup matmul


def _round_up(x, mult):
    return ((x + mult - 1) // mult) * mult


_build_cache = {}


def _build(ncap, mcap):
    """Per-core graph. Inputs (per core):
      qt2  [128, ncap]           bf16   Q^T duplicated: rows 0-63 == rows 64-127
      ktp  [128, npairs*128]     bf16   K^T chunk pairs: pair p = chunk 2p on
                                        partitions 0-63, chunk 2p+1 on 64-127
      vext [128, mchunks*65]     bf16   partition-major Vext chunks: partition
                                        r, cols [c*65:(c+1)*65] = Vext row
                                        c*128+r = [V row | 1.0] (0 if padding)
    Output: out [nblocks*130, W] f32: block j rows [130j:130j+65] = lo half,
      rows [130j+65:130j+130] = hi half; cols [:w_j] = partial
      [ O^T numerator (64 rows) ; denominator (1 row) ] for n-cols j*W..+w_j.
      The host sums lo+hi.
    """
    key = (ncap, mcap)
    if key in _build_cache:
        return _build_cache[key]

    import concourse.bacc as bacc
    import concourse.mybir as mybir
    import concourse.tile as tile

    f32 = mybir.dt.float32
    bf16 = mybir.dt.bfloat16
    i16 = mybir.dt.int16
    mchunks = mcap // 128
    npairs = (mchunks + 1) // 2
    # balanced n-blocks, each <= W (PSUM bank limit)
    nblocks = (ncap + W - 1) // W
    base, extra = divmod(ncap, nblocks)
    widths = [base + (1 if i < extra else 0) for i in range(nblocks)]
    blocks, off = [], 0
    for wd in widths:
        blocks.append((off, wd))
        off += wd
    EXP = mybir.ActivationFunctionType.Exp

    # Schraudolph fast exp on bf16 bit pattern, via int16:
    #   i16 = (int16)(s * (2^7/ln2)/8 + (127*2^7 - C)) ; bitcast -> bf16
    FEXP_A = float(2 ** 7 / np.log(2.0) / 8.0)
    FEXP_B = float(127 * 2 ** 7 - 7.5)

    nc = bacc.Bacc("TRN2", target_bir_lowering=False, debug=False,
                   num_devices=N_CORES)
    qt_d = nc.dram_tensor("qt2", [128, ncap], bf16, kind="ExternalInput")
    ktp_d = nc.dram_tensor("ktp", [128, npairs * 128], bf16,
                           kind="ExternalInput")
    vext_d = nc.dram_tensor("vext", [128, mchunks * 65], bf16,
                            kind="ExternalInput")
    out_d = nc.dram_tensor("out", [len(blocks) * 130, W], f32,
                           kind="ExternalOutput")

    with tile.TileContext(nc) as tc:
        with (
            tc.tile_pool(name="resident", bufs=1) as resident,
            tc.tile_pool(name="pt", bufs=4) as ptp,
            tc.tile_pool(name="osb", bufs=2) as osbp,
            tc.tile_pool(name="psum", bufs=1, space="PSUM") as psum,
        ):
            # sliced DMAs: block-0 slices first so compute starts early
            kt_sb = resident.tile([128, npairs * 128], bf16)
            kcut = min(128, npairs * 128)
            nc.sync.dma_start(kt_sb[:, 0:kcut], ktp_d[:, 0:kcut])
            w0 = blocks[0][1]
            qt_sb = resident.tile([128, ncap], bf16)
            nc.sync.dma_start(qt_sb[:, 0:w0], qt_d[:, 0:w0])
            v_sb = resident.tile([128, mchunks * 65], bf16)
            vcut = min(130, mchunks * 65)
            nc.sync.dma_start(v_sb[:, 0:vcut], vext_d[:, 0:vcut])
            if kcut < npairs * 128:
                nc.sync.dma_start(kt_sb[:, kcut:], ktp_d[:, kcut:])
            if vcut < mchunks * 65:
                nc.sync.dma_start(v_sb[:, vcut:], vext_d[:, vcut:])
            if ncap > w0:
                nc.sync.dma_start(qt_sb[:, w0:], qt_d[:, w0:])

            # PE warmup: keep the HAM activity monitor busy while input DMAs
            # land, so real matmuls run at 2.4 GHz from the start.
            wscr = resident.tile([64, NW_COLS], bf16)
            nc.gpsimd.memset(wscr[:, :], 0)
            for _ in range(N_WARM):
                stw = psum.tile([128, 2, W], f32, tag="st", bufs=3)
                nc.tensor.matmul(stw[0:64, 0, 0:NW_COLS], wscr[:, 0:64],
                                 wscr[:, :], start=True, stop=True,
                                 tile_position=(0, 0))

            pending_drain = None  # (jb, olo, ohi, w) of the previous block

            def drain(jb, olo, ohi, w):
                osl = osbp.tile([65, W], f32, tag="osl")
                nc.vector.tensor_copy(osl[:, 0:w], olo[:, 0:w])
                osh = osbp.tile([65, W], f32, tag="osh")
                nc.scalar.copy(osh[:, 0:w], ohi[:, 0:w])
                nc.sync.dma_start(out_d[jb * 130:jb * 130 + 65, 0:w],
                                  osl[:, 0:w])
                nc.sync.dma_start(out_d[jb * 130 + 65:jb * 130 + 130, 0:w],
                                  osh[:, 0:w])

            def mov(t, half, lo, hi, w):
                ap = t[lo:hi, half, 0:w]
                return ap.bitcast(bf16) if t.dtype == i16 else ap

            parity = 0
            for jb, (j0, w) in enumerate(blocks):
                olo = psum.tile([65, W], f32, tag="olo", bufs=1)
                ohi = psum.tile([65, W], f32, tag="ohi", bufs=1)
                pvq = []  # pending (pt tile, p, has_b), consumed with lag 2

                def pv(t, p, has_b, last):
                    # 64-row contraction halves on alternating row groups:
                    # rows 0-63 accumulate into olo at (0,0), rows 64-127
                    # into ohi at (64,0)
                    for half, mi in ((0, 2 * p), (1, 2 * p + 1)):
                        if half == 1 and not has_b:
                            continue
                        lastc = last and mi == mchunks - 1
                        nc.tensor.matmul(
                            olo[:, 0:w], v_sb[0:64, mi * 65:(mi + 1) * 65],
                            mov(t, half, 0, 64, w), start=(mi == 0),
                            stop=lastc, tile_position=(0, 0),
                            skip_group_check=True)
                        nc.tensor.matmul(
                            ohi[:, 0:w], v_sb[64:128, mi * 65:(mi + 1) * 65],
                            mov(t, half, 64, 128, w), start=(mi == 0),
                            stop=lastc, tile_position=(64, 0),
                            skip_group_check=True)

                for p in range(npairs):
                    has_b = 2 * p + 1 < mchunks
                    # S^T for two m-chunks concurrently (PE row groups) into
                    # one 2-bank PSUM tile
                    st = psum.tile([128, 2, W], f32, tag="st", bufs=3)
                    nc.tensor.matmul(
                        st[:, 0, 0:w], kt_sb[0:64, p * 128:(p + 1) * 128],
                        qt_sb[0:64, j0:j0 + w],
                        start=True, stop=True, tile_position=(0, 0))
                    if has_b:
                        nc.tensor.matmul(
                            st[:, 1, 0:w], kt_sb[64:128, p * 128:(p + 1) * 128],
                            qt_sb[64:128, j0:j0 + w],
                            start=True, stop=True, tile_position=(64, 0))
                    # previous block's output drain: issue before this block's
                    # first exp so olo/ohi free up for pv(pair 0) below
                    if p == 0 and pending_drain is not None:
                        drain(*pending_drain)
                        pending_drain = None
                    # PV runs two pairs behind its exp so the exp result is
                    # already in SBUF when the PE sequencer reaches it: no
                    # semaphore stall, and LDWEIGHTS prefetch keeps flowing
                    if len(pvq) >= 2:
                        pv(*pvq.pop(0), last=False)
                    # P^T = exp(S^T/8), one fused instruction per pair:
                    # exact on ScalarE / fast-exp on VectorE, alternating
                    use_act = (p % 2 == parity)
                    nh = 2 if has_b else 1
                    if use_act:
                        t = ptp.tile([128, 2, W], bf16, tag="pt")
                        nc.scalar.activation(t[:, 0:nh, 0:w], st[:, 0:nh, 0:w],
                                             EXP, scale=0.125)
                    else:
                        t = ptp.tile([128, 2, W], i16, tag="pt")
                        nc.vector.tensor_scalar(
                            t[:, 0:nh, 0:w], st[:, 0:nh, 0:w], FEXP_A, FEXP_B,
                            mybir.AluOpType.mult, mybir.AluOpType.add)
                    pvq.append((t, p, has_b))
                while pvq:
                    job = pvq.pop(0)
                    pv(*job, last=not pvq)
                pending_drain = (jb, olo, ohi, w)
                parity ^= 1
            drain(*pending_drain)

    nc.compile()
    _build_cache[key] = nc
    return nc


def _run(inputs, trace=False):
    queries = np.asarray(inputs["queries"], dtype=np.float32)
    keys = np.asarray(inputs["keys"], dtype=np.float32)
    values = np.asarray(inputs["values"], dtype=np.float32)
    mask_query = np.asarray(inputs["mask_query"])
    mask_key = np.asarray(inputs["mask_key"])

    b, n, d = queries.shape
    dv = values.shape[2]
    assert b == N_CORES, f"batch {b} != {N_CORES} cores"
    bf = ml_dtypes.bfloat16

    idx_q = [np.flatnonzero(mask_query[i]) for i in range(b)]
    idx_k = [np.flatnonzero(mask_key[i]) for i in range(b)]
    ncap = max(max(len(ix) for ix in idx_q), 64)
    mcap = _round_up(max(max(len(ix) for ix in idx_k), 1), 128)
    mchunks = mcap // 128
    npairs = (mchunks + 1) // 2
    nblocks = (ncap + W - 1) // W
    base, extra = divmod(ncap, nblocks)
    bwidths = [base + (1 if i < extra else 0) for i in range(nblocks)]

    qt2 = np.zeros((b, 128, ncap), bf)
    ktp = np.zeros((b, 128, npairs * 128), bf)
    vext = np.zeros((b, 128, mchunks * 65), bf)
    for i in range(b):
        nq, nk = len(idx_q[i]), len(idx_k[i])
        qt2[i, 0:64, :nq] = queries[i, idx_q[i]].T.astype(bf)
        qt2[i, 64:128, :] = qt2[i, 0:64, :]
        kc_t = np.zeros((64, mcap), np.float32)
        kc_t[:, :nk] = keys[i, idx_k[i]].T
        kc_t = kc_t.astype(bf)
        for p in range(npairs):
            ktp[i, 0:64, p * 128:(p + 1) * 128] = \
                kc_t[:, (2 * p) * 128:(2 * p + 1) * 128]
            if 2 * p + 1 < mchunks:
                ktp[i, 64:128, p * 128:(p + 1) * 128] = \
                    kc_t[:, (2 * p + 1) * 128:(2 * p + 2) * 128]
        ve = np.zeros((mcap, 65), np.float32)
        ve[:nk, :dv] = values[i, idx_k[i]]
        ve[:nk, dv] = 1.0
        # partition-major: [chunk, row] -> [row_in_chunk, chunk*65+col]
        vext[i] = ve.reshape(mchunks, 128, 65).transpose(1, 0, 2) \
                    .reshape(128, mchunks * 65).astype(bf)

    nc = _build(ncap, mcap)

    from concourse.bass_utils import run_bass_kernel_spmd
    in_maps = [{"qt2": qt2[i], "ktp": ktp[i], "vext": vext[i]}
               for i in range(b)]
    res = run_bass_kernel_spmd(nc, in_maps, core_ids=list(range(N_CORES)),
                               trace=trace)

    out = np.empty((b, n, dv), np.float32)
    for i in range(b):
        ot = res.results[i]["out"]  # [nblocks*130, W]
        nq, nk = len(idx_q[i]), len(idx_k[i])
        full = np.concatenate(
            [ot[jb * 130:jb * 130 + 65, :bwidths[jb]]
             + ot[jb * 130 + 65:jb * 130 + 130, :bwidths[jb]]
             for jb in range(nblocks)], axis=1)
        num = full[:dv, :nq]
        den = full[dv, :nq]
        if nk > 0:
            out[i, :, :] = values[i, idx_k[i]].mean(axis=0)
        else:
            out[i, :, :] = 0.0
        if nq > 0:
            out[i, idx_q[i], :] = (num / den).T
    return out, res


def kernel(**inputs):
    out, _ = _run(inputs, trace=False)
    return out
